# revision 6
# baseline (speedup 1.0000x reference)
"""3x3 median filter (reflect padding) on Trainium2, 8-core data parallel.

Input  x: (4, 3, 1024, 1024) float32
Output  : (4, 3, 1024, 1024) float32  (Kornia MedianBlur semantics)

Strategy (v4):
  - Host: cast to fp16 (tolerance 2e-2 >> fp16 eps), reflect-pad H/W by 1
    -> (12, 1026, 1026); shard H across 8 cores: core k gets padded rows
    [128k, 128k+130).
  - Device (per core): images processed in groups of G=4 along the free
    dim; all 18 min/max ops of the sorted-column median network run on
    the Vector engine in fp16, which hits the 2x perf mode (2 elem/
    cycle/lane) for every op. Temps are single-buffered (DVE-internal,
    program order serializes them anyway); only the DMA-facing tiles
    (T0-T2 in, out) are double-buffered. Input loads are batched as one
    3D strided DMA per tile per group.
"""

import sys

sys.path.insert(0, "/opt/trn_rl_repo")

import numpy as np

B, C, H, W = 4, 3, 1024, 1024
NIMG = B * C            # 12
NCORES = 8
ROWS_PER_CORE = H // NCORES   # 128
WP = W + 2              # 1026 padded width
HP_CORE = ROWS_PER_CORE + 2   # 130 padded rows per core
G = 4                   # images per op group
NGROUPS = NIMG // G

_PROGRAM = None
LAST_RESULT = None


def _build_program():
    import concourse.bacc as bacc
    import concourse.tile as tile
    import concourse.mybir as mybir
    from contextlib import ExitStack

    f16 = mybir.dt.float16
    mn = mybir.AluOpType.min
    mx = mybir.AluOpType.max

    nc = bacc.Bacc("TRN2", target_bir_lowering=False, debug=False,
                   num_devices=NCORES)
    x = nc.dram_tensor("x", [NIMG, HP_CORE, WP], f16, kind="ExternalInput").ap()
    y = nc.dram_tensor("y", [NIMG, ROWS_PER_CORE, W], f16,
                       kind="ExternalOutput").ap()

    P = ROWS_PER_CORE  # 128 partitions

    with tile.TileContext(nc) as tc, ExitStack() as ctx:
        iop = ctx.enter_context(tc.tile_pool(name="io", bufs=2))
        tp = ctx.enter_context(tc.tile_pool(name="tmp", bufs=1))
        tt = nc.vector.tensor_tensor

        for g in range(NGROUPS):
            T0 = iop.tile([P, G, WP], f16, tag="T0")
            T1 = iop.tile([P, G, WP], f16, tag="T1")
            T2 = iop.tile([P, G, WP], f16, tag="T2")
            i0 = g * G
            # one 3D DMA per tile: dram (G, P, WP) -> sbuf (P, G, WP)
            nc.sync.dma_start(T0[:], x[i0:i0 + G, 0:P, :].transpose([1, 0, 2]))
            nc.sync.dma_start(T1[:], x[i0:i0 + G, 1:P + 1, :].transpose([1, 0, 2]))
            nc.sync.dma_start(T2[:], x[i0:i0 + G, 2:P + 2, :].transpose([1, 0, 2]))

            # vertical sort3 of rows (6 ops); hi reuses M's buffer, mid
            # reuses mm's (in-place second operand is safe: DVE writes
            # lag reads within an instruction).
            m = tp.tile([P, G, WP], f16, tag="m")
            M = tp.tile([P, G, WP], f16, tag="M")
            tt(m[:], T0[:], T1[:], op=mn)
            tt(M[:], T0[:], T1[:], op=mx)
            lo = tp.tile([P, G, WP], f16, tag="lo")
            mm = tp.tile([P, G, WP], f16, tag="mm")
            tt(lo[:], m[:], T2[:], op=mn)
            tt(mm[:], M[:], T2[:], op=mn)
            hi = M
            tt(hi[:], M[:], T2[:], op=mx)
            mid = mm
            tt(mid[:], m[:], mm[:], op=mx)

            # horizontal merge (12 ops). Width-WP pair tiles keep row
            # strides 4B-aligned; only [0:W+1] is valid data.
            pa = tp.tile([P, G, WP], f16, tag="pa")
            pc = tp.tile([P, G, WP], f16, tag="pc")
            pm = tp.tile([P, G, WP], f16, tag="pm")
            pM = tp.tile([P, G, WP], f16, tag="pM")
            tt(pa[:, :, 0:W + 1], lo[:, :, 0:W + 1], lo[:, :, 1:W + 2], op=mx)
            tt(pc[:, :, 0:W + 1], hi[:, :, 0:W + 1], hi[:, :, 1:W + 2], op=mn)
            tt(pm[:, :, 0:W + 1], mid[:, :, 0:W + 1], mid[:, :, 1:W + 2], op=mn)
            tt(pM[:, :, 0:W + 1], mid[:, :, 0:W + 1], mid[:, :, 1:W + 2], op=mx)

            A = tp.tile([P, G, W], f16, tag="A")
            Cm = tp.tile([P, G, W], f16, tag="Cm")
            t2 = tp.tile([P, G, W], f16, tag="t2")
            Bm = tp.tile([P, G, W], f16, tag="Bm")
            tt(A[:], pa[:, :, 0:W], lo[:, :, 2:W + 2], op=mx)
            tt(Cm[:], pc[:, :, 0:W], hi[:, :, 2:W + 2], op=mn)
            tt(t2[:], pM[:, :, 0:W], mid[:, :, 2:W + 2], op=mn)
            tt(Bm[:], pm[:, :, 0:W], t2[:], op=mx)

            m1 = tp.tile([P, G, W], f16, tag="m1")
            M1 = tp.tile([P, G, W], f16, tag="M1")
            tt(m1[:], A[:], Bm[:], op=mn)
            tt(M1[:], A[:], Bm[:], op=mx)
            t3 = M1
            tt(t3[:], M1[:], Cm[:], op=mn)
            out = iop.tile([P, G, W], f16, tag="out")
            tt(out[:], m1[:], t3[:], op=mx)

            nc.sync.dma_start(y[i0:i0 + G].transpose([1, 0, 2]), out[:])

    nc.compile()
    return nc


def _get_program():
    global _PROGRAM
    if _PROGRAM is None:
        _PROGRAM = _build_program()
    return _PROGRAM


def kernel(x):
    global LAST_RESULT
    from concourse.bass_utils import run_bass_kernel_spmd
    import os

    x = np.asarray(x, dtype=np.float32)
    xp = np.pad(x.reshape(NIMG, H, W), ((0, 0), (1, 1), (1, 1)),
                mode="reflect").astype(np.float16)
    in_maps = [
        {"x": np.ascontiguousarray(
            xp[:, ROWS_PER_CORE * k: ROWS_PER_CORE * k + HP_CORE, :])}
        for k in range(NCORES)
    ]
    nc = _get_program()
    trace = bool(int(os.environ.get("MEDIAN_TRACE", "0")))
    res = run_bass_kernel_spmd(nc, in_maps, list(range(NCORES)), trace=trace)
    LAST_RESULT = res
    out = np.concatenate([res.results[k]["y"] for k in range(NCORES)], axis=1)
    return out.reshape(B, C, H, W).astype(np.float32)


# revision 7
# speedup vs baseline: 1.1528x; 1.1528x over previous
"""3x3 median filter (reflect padding) on Trainium2, 8-core data parallel.

Input  x: (4, 3, 1024, 1024) float32
Output  : (4, 3, 1024, 1024) float32  (Kornia MedianBlur semantics)

Strategy (v5):
  - Host: cast to fp16 (tolerance 2e-2 >> fp16 eps), reflect-pad H/W by 1
    -> (12, 1026, 1026); shard H across 8 cores: core k gets padded rows
    [128k, 128k+130).
  - Device (per core): G=6 images concatenated along the free dim into
    ONE flat contiguous stream of L = 6*1026 fp16 per partition. All 18
    min/max ops of the sorted-column median network run on the Vector
    engine over the FULL flat stream: long unit-stride fp16 streams
    engage the DVE's fastest perf mode (~3-3.8 elem/cycle measured),
    while per-image sliced access patterns drop to 1-2x. Horizontal
    shifts just index the flat stream; the 2 positions per image row
    that mix adjacent images compute garbage that lands in lanes the
    output DMA never reads.
  - Aggressive buffer reuse (12 SBUF slots): T0/T1/T2 double-buffered
    for DMA overlap, everything else single-buffered with in-place
    writes where the destination equals the same-position operand.
"""

import sys

sys.path.insert(0, "/opt/trn_rl_repo")

import numpy as np

B, C, H, W = 4, 3, 1024, 1024
NIMG = B * C            # 12
NCORES = 8
ROWS_PER_CORE = H // NCORES   # 128
WP = W + 2              # 1026 padded width
HP_CORE = ROWS_PER_CORE + 2   # 130 padded rows per core
G = 6                   # images per flat group
NGROUPS = NIMG // G     # 2
L = G * WP              # 6156 flat free elems per partition
LPAD = L + 8            # slack so +1/+2 shifted reads stay in bounds

_PROGRAM = None
LAST_RESULT = None


def _build_program():
    import concourse.bacc as bacc
    import concourse.tile as tile
    import concourse.mybir as mybir
    from contextlib import ExitStack

    f16 = mybir.dt.float16
    mn = mybir.AluOpType.min
    mx = mybir.AluOpType.max

    nc = bacc.Bacc("TRN2", target_bir_lowering=False, debug=False,
                   num_devices=NCORES)
    x = nc.dram_tensor("x", [NIMG, HP_CORE, WP], f16, kind="ExternalInput").ap()
    y = nc.dram_tensor("y", [NIMG, ROWS_PER_CORE, W], f16,
                       kind="ExternalOutput").ap()

    P = ROWS_PER_CORE  # 128 partitions

    with tile.TileContext(nc) as tc, ExitStack() as ctx:
        iop = ctx.enter_context(tc.tile_pool(name="io", bufs=2))
        tp = ctx.enter_context(tc.tile_pool(name="tmp", bufs=1))
        tt = nc.vector.tensor_tensor

        for g in range(NGROUPS):
            T0 = iop.tile([P, LPAD], f16, tag="T0")
            T1 = iop.tile([P, LPAD], f16, tag="T1")
            T2 = iop.tile([P, LPAD], f16, tag="T2")
            i0 = g * G
            # one 3D DMA per tile: dram (G, P, WP) -> flat sbuf (P, G*WP)
            nc.sync.dma_start(T0[:, 0:L], x[i0:i0 + G, 0:P, :].transpose([1, 0, 2]))
            nc.sync.dma_start(T1[:, 0:L], x[i0:i0 + G, 1:P + 1, :].transpose([1, 0, 2]))
            nc.sync.dma_start(T2[:, 0:L], x[i0:i0 + G, 2:P + 2, :].transpose([1, 0, 2]))

            # vertical sort3 of rows: lo/mid/hi per column (6 flat ops)
            m = tp.tile([P, LPAD], f16, tag="m")
            M = tp.tile([P, LPAD], f16, tag="M")
            lo = tp.tile([P, LPAD], f16, tag="lo")
            mm = tp.tile([P, LPAD], f16, tag="mm")
            tt(m[:, 0:L], T0[:, 0:L], T1[:, 0:L], op=mn)
            tt(M[:, 0:L], T0[:, 0:L], T1[:, 0:L], op=mx)
            tt(lo[:, 0:L], m[:, 0:L], T2[:, 0:L], op=mn)
            tt(mm[:, 0:L], M[:, 0:L], T2[:, 0:L], op=mn)
            hi = M
            tt(hi[:, 0:L], M[:, 0:L], T2[:, 0:L], op=mx)
            mid = mm
            tt(mid[:, 0:L], m[:, 0:L], mm[:, 0:L], op=mx)

            # horizontal merge, all flat length-L streams; positions that
            # straddle an image boundary produce garbage in lanes the
            # output DMA never reads. T tiles are reused as scratch
            # (dead after the vertical stage).
            pa = T0
            A = T0
            tt(pa[:, 0:L], lo[:, 0:L], lo[:, 1:L + 1], op=mx)
            tt(A[:, 0:L], pa[:, 0:L], lo[:, 2:L + 2], op=mx)
            pc = T1
            Cm = T1
            tt(pc[:, 0:L], hi[:, 0:L], hi[:, 1:L + 1], op=mn)
            tt(Cm[:, 0:L], pc[:, 0:L], hi[:, 2:L + 2], op=mn)
            pm = T2
            pM = m
            tt(pm[:, 0:L], mid[:, 0:L], mid[:, 1:L + 1], op=mn)
            tt(pM[:, 0:L], mid[:, 0:L], mid[:, 1:L + 1], op=mx)
            t2 = m
            tt(t2[:, 0:L], pM[:, 0:L], mid[:, 2:L + 2], op=mn)
            Bm = T2
            tt(Bm[:, 0:L], pm[:, 0:L], t2[:, 0:L], op=mx)

            m1 = lo
            M1 = mm
            tt(m1[:, 0:L], A[:, 0:L], Bm[:, 0:L], op=mn)
            tt(M1[:, 0:L], A[:, 0:L], Bm[:, 0:L], op=mx)
            t3 = M1
            tt(t3[:, 0:L], M1[:, 0:L], Cm[:, 0:L], op=mn)
            out = iop.tile([P, LPAD], f16, tag="out")
            tt(out[:, 0:L], m1[:, 0:L], t3[:, 0:L], op=mx)

            # per-image output DMA: valid columns [0:W) of each row
            for j in range(G):
                nc.sync.dma_start(y[i0 + j], out[:, j * WP:j * WP + W])

    nc.compile()
    return nc


def _get_program():
    global _PROGRAM
    if _PROGRAM is None:
        _PROGRAM = _build_program()
    return _PROGRAM


def kernel(x):
    global LAST_RESULT
    from concourse.bass_utils import run_bass_kernel_spmd
    import os

    x = np.asarray(x, dtype=np.float32)
    xp = np.pad(x.reshape(NIMG, H, W), ((0, 0), (1, 1), (1, 1)),
                mode="reflect").astype(np.float16)
    in_maps = [
        {"x": np.ascontiguousarray(
            xp[:, ROWS_PER_CORE * k: ROWS_PER_CORE * k + HP_CORE, :])}
        for k in range(NCORES)
    ]
    nc = _get_program()
    trace = bool(int(os.environ.get("MEDIAN_TRACE", "0")))
    res = run_bass_kernel_spmd(nc, in_maps, list(range(NCORES)), trace=trace)
    LAST_RESULT = res
    out = np.concatenate([res.results[k]["y"] for k in range(NCORES)], axis=1)
    return out.reshape(B, C, H, W).astype(np.float32)


# revision 10
# speedup vs baseline: 1.1919x; 1.0339x over previous
"""3x3 median filter (reflect padding) on Trainium2, 8-core data parallel.

Input  x: (4, 3, 1024, 1024) float32
Output  : (4, 3, 1024, 1024) float32  (Kornia MedianBlur semantics)

Strategy (v5):
  - Host: cast to fp16 (tolerance 2e-2 >> fp16 eps), reflect-pad H/W by 1
    -> (12, 1026, 1026); shard H across 8 cores: core k gets padded rows
    [128k, 128k+130).
  - Device (per core): G=6 images concatenated along the free dim into
    ONE flat contiguous stream of L = 6*1026 fp16 per partition. All 18
    min/max ops of the sorted-column median network run on the Vector
    engine over the FULL flat stream: long unit-stride fp16 streams
    engage the DVE's fastest perf mode (~3-3.8 elem/cycle measured),
    while per-image sliced access patterns drop to 1-2x. Horizontal
    shifts just index the flat stream; the 2 positions per image row
    that mix adjacent images compute garbage that lands in lanes the
    output DMA never reads.
  - Aggressive buffer reuse (12 SBUF slots): T0/T1/T2 double-buffered
    for DMA overlap, everything else single-buffered with in-place
    writes where the destination equals the same-position operand.
"""

import sys

sys.path.insert(0, "/opt/trn_rl_repo")

import numpy as np

B, C, H, W = 4, 3, 1024, 1024
NIMG = B * C            # 12
NCORES = 8
ROWS_PER_CORE = H // NCORES   # 128
WP = W + 2              # 1026 padded width
HP_CORE = ROWS_PER_CORE + 2   # 130 padded rows per core
GROUPS = [2, 5, 5]      # images per flat group; small first group minimizes
                        # the DMA fill before compute can start, small-ish
                        # last group trims the output-drain tail
GMAX = max(GROUPS)
LMAX = GMAX * WP
LPAD = LMAX + 8         # slack so +1/+2 shifted reads stay in bounds

_PROGRAM = None
LAST_RESULT = None


def _build_program():
    import concourse.bacc as bacc
    import concourse.tile as tile
    import concourse.mybir as mybir
    from contextlib import ExitStack

    f16 = mybir.dt.float16
    mn = mybir.AluOpType.min
    mx = mybir.AluOpType.max

    nc = bacc.Bacc("TRN2", target_bir_lowering=False, debug=False,
                   num_devices=NCORES)
    x = nc.dram_tensor("x", [NIMG, HP_CORE, WP], f16, kind="ExternalInput").ap()
    y = nc.dram_tensor("y", [NIMG, ROWS_PER_CORE, W], f16,
                       kind="ExternalOutput").ap()

    P = ROWS_PER_CORE  # 128 partitions

    with tile.TileContext(nc) as tc, ExitStack() as ctx:
        iop = ctx.enter_context(tc.tile_pool(name="io", bufs=2))
        tp = ctx.enter_context(tc.tile_pool(name="tmp", bufs=1))
        tt = nc.vector.tensor_tensor

        i0 = 0
        for G in GROUPS:
            L = G * WP
            T0 = iop.tile([P, LPAD], f16, tag="T0")
            T1 = iop.tile([P, LPAD], f16, tag="T1")
            T2 = iop.tile([P, LPAD], f16, tag="T2")
            # one 3D DMA per tile: dram (G, P, WP) -> flat sbuf (P, G*WP)
            nc.sync.dma_start(T0[:, 0:L], x[i0:i0 + G, 0:P, :].transpose([1, 0, 2]))
            nc.sync.dma_start(T1[:, 0:L], x[i0:i0 + G, 1:P + 1, :].transpose([1, 0, 2]))
            nc.sync.dma_start(T2[:, 0:L], x[i0:i0 + G, 2:P + 2, :].transpose([1, 0, 2]))

            # vertical sort3 of rows: lo/mid/hi per column (6 flat ops)
            m = tp.tile([P, LPAD], f16, tag="m")
            M = tp.tile([P, LPAD], f16, tag="M")
            lo = tp.tile([P, LPAD], f16, tag="lo")
            mm = tp.tile([P, LPAD], f16, tag="mm")
            tt(m[:, 0:L], T0[:, 0:L], T1[:, 0:L], op=mn)
            tt(M[:, 0:L], T0[:, 0:L], T1[:, 0:L], op=mx)
            tt(lo[:, 0:L], m[:, 0:L], T2[:, 0:L], op=mn)
            tt(mm[:, 0:L], M[:, 0:L], T2[:, 0:L], op=mn)
            hi = M
            tt(hi[:, 0:L], M[:, 0:L], T2[:, 0:L], op=mx)
            mid = mm
            tt(mid[:, 0:L], m[:, 0:L], mm[:, 0:L], op=mx)

            # horizontal merge, all flat length-L streams; positions that
            # straddle an image boundary produce garbage in lanes the
            # output DMA never reads. T tiles are reused as scratch
            # (dead after the vertical stage).
            pa = T0
            A = T0
            tt(pa[:, 0:L], lo[:, 0:L], lo[:, 1:L + 1], op=mx)
            tt(A[:, 0:L], pa[:, 0:L], lo[:, 2:L + 2], op=mx)
            pc = T1
            Cm = T1
            tt(pc[:, 0:L], hi[:, 0:L], hi[:, 1:L + 1], op=mn)
            tt(Cm[:, 0:L], pc[:, 0:L], hi[:, 2:L + 2], op=mn)
            pm = T2
            pM = m
            tt(pm[:, 0:L], mid[:, 0:L], mid[:, 1:L + 1], op=mn)
            tt(pM[:, 0:L], mid[:, 0:L], mid[:, 1:L + 1], op=mx)
            t2 = m
            tt(t2[:, 0:L], pM[:, 0:L], mid[:, 2:L + 2], op=mn)
            Bm = T2
            tt(Bm[:, 0:L], pm[:, 0:L], t2[:, 0:L], op=mx)

            m1 = lo
            M1 = mm
            tt(m1[:, 0:L], A[:, 0:L], Bm[:, 0:L], op=mn)
            tt(M1[:, 0:L], A[:, 0:L], Bm[:, 0:L], op=mx)
            t3 = M1
            tt(t3[:, 0:L], M1[:, 0:L], Cm[:, 0:L], op=mn)
            out = iop.tile([P, LPAD], f16, tag="out")
            tt(out[:, 0:L], m1[:, 0:L], t3[:, 0:L], op=mx)

            # per-image output DMA: valid columns [0:W) of each row
            for j in range(G):
                nc.sync.dma_start(y[i0 + j], out[:, j * WP:j * WP + W])
            i0 += G

    nc.compile()
    return nc


def _get_program():
    global _PROGRAM
    if _PROGRAM is None:
        _PROGRAM = _build_program()
    return _PROGRAM


def kernel(x):
    global LAST_RESULT
    from concourse.bass_utils import run_bass_kernel_spmd
    import os

    x = np.asarray(x, dtype=np.float32)
    xp = np.pad(x.reshape(NIMG, H, W), ((0, 0), (1, 1), (1, 1)),
                mode="reflect").astype(np.float16)
    in_maps = [
        {"x": np.ascontiguousarray(
            xp[:, ROWS_PER_CORE * k: ROWS_PER_CORE * k + HP_CORE, :])}
        for k in range(NCORES)
    ]
    nc = _get_program()
    trace = bool(int(os.environ.get("MEDIAN_TRACE", "0")))
    res = run_bass_kernel_spmd(nc, in_maps, list(range(NCORES)), trace=trace)
    LAST_RESULT = res
    out = np.concatenate([res.results[k]["y"] for k in range(NCORES)], axis=1)
    return out.reshape(B, C, H, W).astype(np.float32)


# revision 13
# speedup vs baseline: 1.2026x; 1.0090x over previous
"""3x3 median filter (reflect padding) on Trainium2, 8-core data parallel.

Input  x: (4, 3, 1024, 1024) float32
Output  : (4, 3, 1024, 1024) float32  (Kornia MedianBlur semantics)

Strategy (v5):
  - Host: cast to fp16 (tolerance 2e-2 >> fp16 eps), reflect-pad H/W by 1
    -> (12, 1026, 1026); shard H across 8 cores: core k gets padded rows
    [128k, 128k+130).
  - Device (per core): G=6 images concatenated along the free dim into
    ONE flat contiguous stream of L = 6*1026 fp16 per partition. All 18
    min/max ops of the sorted-column median network run on the Vector
    engine over the FULL flat stream: long unit-stride fp16 streams
    engage the DVE's fastest perf mode (~3-3.8 elem/cycle measured),
    while per-image sliced access patterns drop to 1-2x. Horizontal
    shifts just index the flat stream; the 2 positions per image row
    that mix adjacent images compute garbage that lands in lanes the
    output DMA never reads.
  - Aggressive buffer reuse (12 SBUF slots): T0/T1/T2 double-buffered
    for DMA overlap, everything else single-buffered with in-place
    writes where the destination equals the same-position operand.
"""

import sys

sys.path.insert(0, "/opt/trn_rl_repo")

import numpy as np

B, C, H, W = 4, 3, 1024, 1024
NIMG = B * C            # 12
NCORES = 8
ROWS_PER_CORE = H // NCORES   # 128
WP = W + 2              # 1026 padded width
HP_CORE = ROWS_PER_CORE + 2   # 130 padded rows per core
GROUPS = [2, 5, 5]      # images per flat group; small first group minimizes
                        # the DMA fill before compute can start, small-ish
                        # last group trims the output-drain tail
GMAX = max(GROUPS)
LMAX = GMAX * WP
LPAD = LMAX + 8         # slack so +1/+2 shifted reads stay in bounds

_PROGRAM = None
LAST_RESULT = None


def _build_program():
    import concourse.bacc as bacc
    import concourse.tile as tile
    import concourse.mybir as mybir
    from contextlib import ExitStack

    f16 = mybir.dt.float16
    mn = mybir.AluOpType.min
    mx = mybir.AluOpType.max

    nc = bacc.Bacc("TRN2", target_bir_lowering=False, debug=False,
                   num_devices=NCORES)
    x = nc.dram_tensor("x", [NIMG, HP_CORE, WP], f16, kind="ExternalInput").ap()
    y = nc.dram_tensor("y", [NIMG, ROWS_PER_CORE, W], f16,
                       kind="ExternalOutput").ap()

    P = ROWS_PER_CORE  # 128 partitions

    with tile.TileContext(nc) as tc, ExitStack() as ctx:
        iop = ctx.enter_context(tc.tile_pool(name="io", bufs=2))
        tp = ctx.enter_context(tc.tile_pool(name="tmp", bufs=1))
        tt = nc.vector.tensor_tensor

        i0 = 0
        for gi, G in enumerate(GROUPS):
            L = G * WP
            T0 = iop.tile([P, LPAD], f16, tag="T0")
            T1 = iop.tile([P, LPAD], f16, tag="T1")
            T2 = iop.tile([P, LPAD], f16, tag="T2")
            # one 3D DMA per tile: dram (G, P, WP) -> flat sbuf (P, G*WP).
            # T1 goes through the scalar engine's queue so the first two
            # loads (all that the first op needs) transfer in parallel.
            nc.sync.dma_start(T0[:, 0:L], x[i0:i0 + G, 0:P, :].transpose([1, 0, 2]))
            nc.scalar.dma_start(T1[:, 0:L], x[i0:i0 + G, 1:P + 1, :].transpose([1, 0, 2]))
            nc.sync.dma_start(T2[:, 0:L], x[i0:i0 + G, 2:P + 2, :].transpose([1, 0, 2]))

            # vertical sort3 of rows: lo/mid/hi per column (6 flat ops)
            m = tp.tile([P, LPAD], f16, tag="m")
            M = tp.tile([P, LPAD], f16, tag="M")
            lo = tp.tile([P, LPAD], f16, tag="lo")
            mm = tp.tile([P, LPAD], f16, tag="mm")
            tt(m[:, 0:L], T0[:, 0:L], T1[:, 0:L], op=mn)
            tt(M[:, 0:L], T0[:, 0:L], T1[:, 0:L], op=mx)
            tt(lo[:, 0:L], m[:, 0:L], T2[:, 0:L], op=mn)
            tt(mm[:, 0:L], M[:, 0:L], T2[:, 0:L], op=mn)
            hi = M
            tt(hi[:, 0:L], M[:, 0:L], T2[:, 0:L], op=mx)
            mid = mm
            tt(mid[:, 0:L], m[:, 0:L], mm[:, 0:L], op=mx)

            # horizontal merge, all flat length-L streams; positions that
            # straddle an image boundary produce garbage in lanes the
            # output DMA never reads. T tiles are reused as scratch
            # (dead after the vertical stage).
            pa = T0
            A = T0
            tt(pa[:, 0:L], lo[:, 0:L], lo[:, 1:L + 1], op=mx)
            tt(A[:, 0:L], pa[:, 0:L], lo[:, 2:L + 2], op=mx)
            pc = T1
            Cm = T1
            tt(pc[:, 0:L], hi[:, 0:L], hi[:, 1:L + 1], op=mn)
            tt(Cm[:, 0:L], pc[:, 0:L], hi[:, 2:L + 2], op=mn)
            pm = T2
            pM = m
            tt(pm[:, 0:L], mid[:, 0:L], mid[:, 1:L + 1], op=mn)
            tt(pM[:, 0:L], mid[:, 0:L], mid[:, 1:L + 1], op=mx)
            t2 = m
            tt(t2[:, 0:L], pM[:, 0:L], mid[:, 2:L + 2], op=mn)
            Bm = T2
            tt(Bm[:, 0:L], pm[:, 0:L], t2[:, 0:L], op=mx)

            m1 = lo
            M1 = mm
            tt(m1[:, 0:L], A[:, 0:L], Bm[:, 0:L], op=mn)
            tt(M1[:, 0:L], A[:, 0:L], Bm[:, 0:L], op=mx)
            t3 = M1
            tt(t3[:, 0:L], M1[:, 0:L], Cm[:, 0:L], op=mn)
            out = iop.tile([P, LPAD], f16, tag="out")
            if gi == len(GROUPS) - 1:
                # last group: per-image final op + DMA so the output
                # transfers overlap the remaining compute instead of
                # draining after the last instruction
                for j in range(G):
                    s = j * WP
                    tt(out[:, s:s + W], m1[:, s:s + W], t3[:, s:s + W], op=mx)
                    nc.sync.dma_start(y[i0 + j], out[:, s:s + W])
            else:
                tt(out[:, 0:L], m1[:, 0:L], t3[:, 0:L], op=mx)
                for j in range(G):
                    nc.sync.dma_start(y[i0 + j], out[:, j * WP:j * WP + W])
            i0 += G

    nc.compile()
    return nc


def _get_program():
    global _PROGRAM
    if _PROGRAM is None:
        _PROGRAM = _build_program()
    return _PROGRAM


def kernel(x):
    global LAST_RESULT
    from concourse.bass_utils import run_bass_kernel_spmd
    import os

    x = np.asarray(x, dtype=np.float32)
    xp = np.pad(x.reshape(NIMG, H, W), ((0, 0), (1, 1), (1, 1)),
                mode="reflect").astype(np.float16)
    in_maps = [
        {"x": np.ascontiguousarray(
            xp[:, ROWS_PER_CORE * k: ROWS_PER_CORE * k + HP_CORE, :])}
        for k in range(NCORES)
    ]
    nc = _get_program()
    trace = bool(int(os.environ.get("MEDIAN_TRACE", "0")))
    res = run_bass_kernel_spmd(nc, in_maps, list(range(NCORES)), trace=trace)
    LAST_RESULT = res
    out = np.concatenate([res.results[k]["y"] for k in range(NCORES)], axis=1)
    return out.reshape(B, C, H, W).astype(np.float32)
